# revision 9
# baseline (speedup 1.0000x reference)
"""BiLSTM(2-layer) + CRF NLL Trainium2 kernel, v2: direction-split sharding.

8 cores = 4 pairs. Pair p owns 16 sequences; core 2p runs the FORWARD
direction of both LSTM layers for those 16 sequences, core 2p+1 the BACKWARD
direction. Backward cores see time-reversed inputs, so every core runs an
identical forward-scan program; all direction asymmetry lives in host staging
(weights, reversed inputs, transposed CRF transitions, swapped start/end).

Between layers the pair exchanges hidden states with a 2-core AllGather
(bounce via DRAM, sent time-reversed so the partner receives data in its own
time order); each core reconstructs the partner's h via
(slot0 + slot1) - own, computed in fp32 so the bf16 cancellation is exact.
Emissions are per-direction partials pair-summed the same way. Each core then
runs the CRF on all 16 sequences and masks the per-sequence losses so each
sequence is counted on exactly one core.

vs v1: the serial recurrent matmul chain per core drops from 36864 LDW+MM
pairs (N=8) to 18360 (N=16), and input-projection/emission matmuls are
interleaved into the recurrence as PE filler during the per-step elementwise
tails. Gate chunks are reordered [g, i, f, o] so tanh(g)/sigmoid(i,f) start
before the step's matmuls finish and only sigmoid(o) trails them.
"""

import sys
import numpy as np
import ml_dtypes

sys.path.insert(0, "/opt/trn_rl_repo")

import concourse.bass as bass
import concourse.mybir as mybir
import concourse.tile as tile

dt = mybir.dt
AF = mybir.ActivationFunctionType
bf16 = ml_dtypes.bfloat16

# problem constants
B, T, E, H, K = 64, 256, 768, 384, 9
NC = 8
BL = 16         # sequences per core (one direction)
G = 4 * H       # 1536
NE = 6          # input contract chunks (768/128, both layers)
NH = H // 128   # 3
NG = G // 128   # 12
BLK = 32        # timesteps per xg block
NB = T // BLK   # 8
NQ = T * BL     # 4096
CRF_S = 8

# gate chunk order [i, f, g, o] = native pytorch order. The step's matmuls
# run as three PSUM-bank groups (i+f, g, o) so each activation starts as soon
# as its bank's accumulation retires, overlapping the rest of the matmul
# stream; o last so only sigmoid(o) + one mul trail the final matmul.
GATE_PERM = np.arange(4 * H)


def split_waits(nc):
    """Legalize sem waits: walrus accepts at most one sync wait per
    instruction; hoist extra waits onto same-engine NoOps."""
    import bass_rust

    n_split = 0
    for f in nc.m.functions:
        for blk in f.blocks:
            out = []
            changed = False
            for inst in blk.instructions:
                si = inst.sync_info
                if si is not None and si.on_wait and len(si.on_wait) > 1:
                    waits = list(si.on_wait)
                    for k, w in enumerate(waits[:-1]):
                        nop = mybir.InstNoOp(name=f"{inst.name}_w{k}", ins=[], outs=[])
                        nop.engine = inst.engine
                        nop.sync_info = bass_rust.SyncInfo(on_wait=[w], on_update=[])
                        out.append(nop)
                        n_split += 1
                    inst.sync_info = bass_rust.SyncInfo(
                        on_wait=[waits[-1]], on_update=list(si.on_update or [])
                    )
                    changed = True
                out.append(inst)
            if changed:
                blk.instructions = out
    return n_split


def rev_slice(a, b):
    """slice covering [a, b) traversed in reverse order."""
    return slice(b - 1, None if a == 0 else a - 1, -1)


def build_nc(legalize=True):
    nc = bass.Bass(trn_type="TRN2", num_devices=NC)
    f32 = dt.float32
    groups = [[2 * p, 2 * p + 1] for p in range(NC // 2)]

    xT_d = nc.declare_dram_parameter("xT", [NE, 128, T, BL], dt.bfloat16, False)
    w0_d = nc.declare_dram_parameter("w0T", [NE, 128, G], dt.bfloat16, False)
    w1_d = nc.declare_dram_parameter("w1T", [NE, 128, G], dt.bfloat16, False)
    whh_d = nc.declare_dram_parameter("whhT", [2, NH, 128, G], dt.bfloat16, False)
    bias_d = nc.declare_dram_parameter("bias", [128, 2 * NG], f32, False)
    wout_d = nc.declare_dram_parameter("woutT", [NH, 128, K], dt.bfloat16, False)
    bout_d = nc.declare_dram_parameter("bout", [K, 1], f32, False)
    oh_d = nc.declare_dram_parameter("ohT", [K, T, BL], f32, False)
    id_d = nc.declare_dram_parameter("ident", [128, 128], dt.bfloat16, False)
    crf_d = nc.declare_dram_parameter("crf", [K, 32], f32, False)
    lmask_d = nc.declare_dram_parameter("lmask", [1, BL], f32, False)
    loss_d = nc.declare_dram_parameter("loss", [1, 1], f32, True)

    RB = 2 * BLK  # h1 ring length (timesteps); emissions drain a block behind

    with tile.TileContext(nc) as tc:
        with (
            tc.tile_pool(name="big", bufs=1) as big,
            tc.tile_pool(name="state", bufs=2) as state,
            tc.tile_pool(name="tmp", bufs=3) as tmp,
            tc.tile_pool(name="xgp", bufs=1) as xgp,
            tc.tile_pool(name="ps", bufs=2, space="PSUM") as ps,
            tc.tile_pool(name="dram", bufs=1, space="DRAM") as dram,
        ):
            # ---- persistent loads ----
            xT = big.tile([128, NE, T, BL], dt.bfloat16, tag="xT")
            for ch in range(NE):
                nc.sync.dma_start(xT[:, ch], xT_d[ch])
            bias = big.tile([128, 2 * NG], f32, tag="bias")
            nc.sync.dma_start(bias[:], bias_d[:])
            wout = big.tile([128, NH, K], dt.bfloat16, tag="wout")
            for ch in range(NH):
                nc.sync.dma_start(wout[:, ch], wout_d[ch])
            bout = big.tile([K, 1], f32, tag="bout")
            nc.sync.dma_start(bout[:], bout_d[:])
            ident = big.tile([128, 128], dt.bfloat16, tag="ident")
            nc.sync.dma_start(ident[:], id_d[:])
            # stage via DVE copies (single-DMA-queue-consumer rule)
            ohT_raw = big.tile([K, T, BL], f32, tag="em")  # slot later: u, em
            nc.sync.dma_start(ohT_raw[:], oh_d[:])
            ohT = big.tile([K, T, BL], f32, tag="ohT")
            nc.vector.tensor_copy(ohT[:], ohT_raw[:])
            crf_raw = big.tile([K, 32], f32, tag="crf_raw")
            nc.sync.dma_start(crf_raw[:], crf_d[:])
            crf = big.tile([K, 32], f32, tag="crf")
            nc.vector.tensor_copy(crf[:], crf_raw[:])
            lmask_raw = big.tile([1, BL], f32, tag="lmask_raw")
            nc.sync.dma_start(lmask_raw[:], lmask_d[:])
            lmask = big.tile([1, BL], f32, tag="lmask")
            nc.vector.tensor_copy(lmask[:], lmask_raw[:])

            h0 = big.tile([128, NH, T, BL], dt.bfloat16, tag="h0")
            h1r = big.tile([128, NH, RB, BL], dt.bfloat16, tag="h1r")

            # DRAM bounce buffers for the pairwise exchanges. h0 is exchanged
            # in NS pipelined segments, each fired as soon as its blocks are
            # computed; only the last segment's latency is exposed at the
            # layer boundary.
            NS = 4
            SEG = T // NS
            b0_in = [
                dram.tile([128, NH, SEG, BL], dt.bfloat16, tag=f"b0_in{s}",
                          name=f"b0_in{s}")
                for s in range(NS)
            ]
            b0_out = [
                nc.dram_tensor(f"b0_out{s}", [2, 128, NH, SEG, BL], dt.bfloat16)
                for s in range(NS)
            ]
            b1_in = dram.tile([K, T, BL], f32, tag="b1_in")
            b1_out = nc.dram_tensor("b1_out", [2, K, T, BL], f32)

            em = None  # allocated after the h0 exchange (shares slot with u)

            # numerator transition-pairs scratch: filled one tile per layer-0
            # block (depends only on ohT/crf), reduced at layer-0 end
            oh_flat = ohT[:].rearrange("k t b -> k (t b)")
            NTC = 512
            NQm = NQ - BL
            scr2 = big.tile([K, BL, T], f32, tag="XC")
            scr2_tb = scr2[:].rearrange("k b t -> k t b")  # [K, T, BL]
            tr_t = tmp.tile([K, BL], f32, tag="trt")

            def pairs_task(nt):
                n0 = nt * NTC
                n1 = min(n0 + NTC, NQm)
                pa = ps.tile([K, NTC], f32, tag="misc", bufs=1, name=f"pa{nt}")
                nc.tensor.matmul(pa[:, 0:n1 - n0], crf[:, 0:K], oh_flat[:, n0:n1],
                                 start=True, stop=True)
                nc.vector.tensor_tensor(
                    scr2_tb[:, n0 // BL:n1 // BL, :],
                    pa[:, 0:n1 - n0], oh_flat[:, n0 + BL:n1 + BL],
                    mybir.AluOpType.mult,
                )
                if nt == NB - 1:
                    nc.vector.tensor_reduce(
                        tr_t[:], scr2[:, :, 0:T - 1], mybir.AxisListType.X,
                        mybir.AluOpType.add,
                    )

            class XgEmitter:
                """Incrementally emits the input-projection matmuls for one
                32-step block (12 gate chunks x 6 contract chunks) so they can
                be interleaved into the recurrence as PE filler."""

                def __init__(self, layer, blk, xg_tile, wih):
                    self.layer, self.blk, self.xg, self.wih = layer, blk, xg_tile, wih
                    self.j, self.kc, self.p = 0, 0, None

                def rhs(self, kc):
                    sl = slice(self.blk * BLK, (self.blk + 1) * BLK)
                    if self.layer == 0:
                        return xT[:, kc, sl, :]
                    if kc < NH:
                        return h0[:, kc, sl, :]
                    return xT[:, kc - NH, sl, :]  # partner h0 lives in xT[:, 0:3]

                def step(self):
                    if self.j >= NG:
                        return False
                    if self.kc == 0:
                        self.p = ps.tile([128, BLK * BL], dt.float32, tag="pxg", bufs=2)
                    j = self.j
                    nc.tensor.matmul(
                        self.p[:],
                        self.wih[:, self.kc, j * 128:(j + 1) * 128],
                        self.rhs(self.kc),
                        start=(self.kc == 0),
                        stop=(self.kc == NE - 1),
                    )
                    self.kc += 1
                    if self.kc == NE:
                        bcol = self.layer * NG + j
                        nc.scalar.add(self.xg[:, j], self.p[:], bias[:, bcol:bcol + 1])
                        self.kc = 0
                        self.j += 1
                    return True

                def drain(self):
                    while self.step():
                        pass

            # ---- two LSTM layers (one direction each; SPMD over cores) ----
            for layer in range(2):
                wih = big.tile([128, NE, G], dt.bfloat16, tag="wih")
                w_src = w0_d if layer == 0 else w1_d
                for ch in range(NE):
                    nc.sync.dma_start(wih[:, ch], w_src[ch])
                whh = big.tile([128, NH, G], dt.bfloat16, tag="whh")
                for kc in range(NH):
                    nc.sync.dma_start(whh[:, kc], whh_d[layer, kc])

                if layer == 1:
                    em = big.tile([K, T, BL], f32, tag="em")

                def h_chunk(t, kc):
                    if layer == 0:
                        return h0[:, kc, t, :]
                    return h1r[:, kc, t % RB, :]

                def h_full(t):
                    if layer == 0:
                        return h0[:, :, t, :]
                    return h1r[:, :, t % RB, :]

                xg_cur = xgp.tile([128, NG, BLK * BL], dt.bfloat16, tag="xg", bufs=2)
                em0 = XgEmitter(layer, 0, xg_cur, wih)
                em0.drain()

                c_st = None
                for blk in range(NB):
                    if blk + 1 < NB:
                        xg_nxt = xgp.tile(
                            [128, NG, BLK * BL], dt.bfloat16, tag="xg", bufs=2
                        )
                        nxt = XgEmitter(layer, blk + 1, xg_nxt, wih)
                    else:
                        xg_nxt, nxt = None, None

                    for tl in range(BLK):
                        t = blk * BLK + tl
                        first = t == 0
                        u0 = tl * BL

                        # Gate pre-activations land in three separate PSUM
                        # banks (i+f, g, o). Each bank's group: recurrent
                        # whh matmuls plus one identity-matmul per gate chunk
                        # that injects xg (incl. bias) straight into PSUM —
                        # no DVE pre-adds, and each activation reads its bank
                        # as soon as that group retires while the PE streams
                        # the next group.
                        gp_if = ps.tile([128, 2 * NH, BL], f32, tag="gp_if", bufs=1)
                        gp_g = ps.tile([128, NH, BL], f32, tag="gp_g", bufs=1)
                        gp_o = ps.tile([128, NH, BL], f32, tag="gp_o", bufs=1)

                        def emit_group(tile_, j0, nj):
                            # ONE identity matmul FIRST (start=True): it has
                            # no dependency on h(t-1), so it issues during the
                            # previous step's elementwise tail and injects xg
                            # for the whole group; the nj*NH recurrent matmuls
                            # then accumulate on top. kc-outer order: the
                            # first matmuls need only h chunk 0, which the
                            # chunk-split hmul below writes first.
                            total = (0 if first else nj * NH) + 1
                            nc.tensor.matmul(
                                tile_[:],
                                ident[:],
                                xg_cur[:, j0:j0 + nj, u0:u0 + BL],
                                start=True,
                                stop=(total == 1),
                            )
                            n = 1
                            if not first:
                                for jj in range(nj):
                                    j = j0 + jj
                                    for kc in range(NH):
                                        nc.tensor.matmul(
                                            tile_[:, jj],
                                            whh[:, kc, j * 128:(j + 1) * 128],
                                            h_chunk(t - 1, kc),
                                            start=False,
                                            stop=(n == total - 1),
                                        )
                                        n += 1

                        emit_group(gp_if, 0, 2 * NH)
                        emit_group(gp_g, 2 * NH, NH)
                        emit_group(gp_o, 3 * NH, NH)

                        sif = tmp.tile([128, 2 * NH, BL], f32, tag="sif")
                        nc.scalar.activation(sif[:], gp_if[:], AF.Sigmoid)
                        tg = tmp.tile([128, NH, BL], f32, tag="tg")
                        nc.scalar.activation(tg[:], gp_g[:], AF.Tanh)
                        so = tmp.tile([128, NH, BL], f32, tag="so")
                        nc.scalar.activation(so[:], gp_o[:], AF.Sigmoid)

                        cN = state.tile([128, NH, BL], f32, tag="c")
                        if first:
                            nc.vector.tensor_mul(cN[:], sif[:, 0:NH], tg[:])
                        else:
                            t2 = tmp.tile([128, NH, BL], f32, tag="t2")
                            nc.vector.tensor_mul(t2[:], sif[:, NH:2 * NH], c_st[:])
                            t1 = tmp.tile([128, NH, BL], f32, tag="t1")
                            nc.vector.tensor_mul(t1[:], sif[:, 0:NH], tg[:])
                            nc.vector.tensor_add(cN[:], t1[:], t2[:])
                        c_st = cN

                        tc_t = tmp.tile([128, NH, BL], f32, tag="tc")
                        nc.scalar.activation(tc_t[:], cN[:], AF.Tanh)
                        nc.vector.tensor_mul(h_full(t), so[:], tc_t[:])

                        # PE filler: next block's input projections, spread
                        # evenly so every step's elementwise tail is covered
                        # (72 matmuls over 32 steps = 2-3 per step)
                        if nxt is not None:
                            nmm = NG * NE
                            quota = ((tl + 1) * nmm) // BLK - (tl * nmm) // BLK
                            for _ in range(quota):
                                nxt.step()

                    if nxt is not None:
                        nxt.drain()
                        xg_cur = xg_nxt

                    if layer == 0:
                        # send this h0 block time-reversed into its segment
                        # bounce (per chunk: DMA APs are limited to 3 dims)
                        seg = blk // (BLK_PER_SEG := NB // NS)
                        t1r = SEG * (seg + 1) - blk * BLK
                        rsl = rev_slice(t1r - BLK, t1r)
                        for c in range(NH):
                            nc.sync.dma_start(
                                b0_in[seg][:, c, rsl, :],
                                h0[:, c, blk * BLK:(blk + 1) * BLK, :],
                            )
                        pairs_task(blk)
                        if blk % BLK_PER_SEG == BLK_PER_SEG - 1 and seg < NS - 1:
                            # segment collective fires as soon as its blocks
                            # are sent; transfer hides under remaining compute
                            nc.gpsimd.collective_compute(
                                "AllGather",
                                mybir.AluOpType.bypass,
                                replica_groups=groups,
                                ins=[b0_in[seg][:].opt()],
                                outs=[b0_out[seg][:].opt()],
                            )
                    else:
                        # emissions for the ring block just completed
                        r0 = (blk % 2) * BLK
                        pem = ps.tile([K, BLK, BL], f32, tag="misc", bufs=1)
                        for kc in range(NH):
                            nc.tensor.matmul(
                                pem[:],
                                wout[:, kc],
                                h1r[:, kc, r0:r0 + BLK, :],
                                start=(kc == 0),
                                stop=(kc == NH - 1),
                            )
                        nc.scalar.add(
                            em[:, blk * BLK:(blk + 1) * BLK, :], pem[:], bout[:, 0:1]
                        )

                if layer == 0:
                    # ---- last h0 segment exchange (only this one's latency
                    # is exposed) ----
                    nc.gpsimd.collective_compute(
                        "AllGather",
                        mybir.AluOpType.bypass,
                        replica_groups=groups,
                        ins=[b0_in[NS - 1][:].opt()],
                        outs=[b0_out[NS - 1][:].opt()],
                    )
                    # partner h0 = (slot0 + slot1) - own(reversed); fp32 sum
                    # makes the bf16 cancellation exact. Descending segments:
                    # the last segment holds partner t 0.., consumed first.
                    for s in range(NS - 1, -1, -1):
                        lo = T - SEG * (s + 1)
                        for c in range(NH):
                            s0c = big.tile([128, SEG, BL], dt.bfloat16, tag="XC")
                            nc.sync.dma_start(s0c[:], b0_out[s][0, :, c])
                            s1c = big.tile([128, SEG, BL], dt.bfloat16, tag="XD")
                            nc.sync.dma_start(s1c[:], b0_out[s][1, :, c])
                            u = big.tile([128, SEG, BL], f32, tag="em")
                            nc.vector.tensor_add(u[:], s0c[:], s1c[:])
                            nc.vector.tensor_sub(
                                xT[:, c, lo:lo + SEG, :], u[:],
                                h0[:, c, rev_slice(T - lo - SEG, T - lo), :],
                            )

            # ---- emissions exchange: em_full = own partial + partner partial ----
            nc.sync.dma_start(b1_in[:, ::-1, :], em[:])
            nc.gpsimd.collective_compute(
                "AllGather",
                mybir.AluOpType.bypass,
                replica_groups=groups,
                ins=[b1_in[:].opt()],
                outs=[b1_out[:].opt()],
            )
            s0e = big.tile([K, T, BL], f32, tag="wih")
            nc.sync.dma_start(s0e[:], b1_out[0])
            s1e = big.tile([K, T, BL], f32, tag="XC")
            nc.sync.dma_start(s1e[:], b1_out[1])
            nc.vector.tensor_add(s0e[:], s0e[:], s1e[:])
            # em_full = own + partner. The slot sum minus our (reversed) send
            # gives the PARTNER partial in our time order; our own partial
            # must be added back. (Written to a fresh tile: a reversed
            # self-read in one op would race an in-place write.)
            em2 = big.tile([K, T, BL], f32, tag="whh")
            nc.vector.tensor_sub(em2[:], s0e[:], em[:, ::-1, :])
            nc.vector.tensor_add(em2[:], em2[:], em[:])
            em = em2

            # ---- gold path score (numerator) ----
            scr = big.tile([K, BL, T], f32, tag="wih")
            nkb = tmp.tile([K, BL], f32, tag="nkb")
            nc.vector.tensor_tensor(
                scr[:].rearrange("k b t -> k t b"),
                em[:], ohT[:], mybir.AluOpType.mult,
            )
            nc.vector.tensor_reduce(
                nkb[:], scr[:], mybir.AxisListType.X, mybir.AluOpType.add
            )
            nc.vector.tensor_add(nkb[:], nkb[:], tr_t[:])
            pnum = ps.tile([1, BL], f32, tag="misc", bufs=1)
            nc.tensor.matmul(pnum[:], crf[:, 22:23], nkb[:], start=True, stop=False)
            nc.tensor.matmul(pnum[:], crf[:, 20:21], ohT[:, 0, :], start=False, stop=False)
            nc.tensor.matmul(pnum[:], crf[:, 21:22], ohT[:, T - 1, :], start=False, stop=True)
            num = tmp.tile([1, BL], f32, tag="num")
            nc.vector.tensor_copy(num[:], pnum[:])

            # ---- CRF forward algorithm (denominator), linear space ----
            # two independent 8-sequence chains interleaved so each chain's
            # PE->DVE latency hides under the other's ops
            eem = big.tile([K, T, BL], f32, tag="wih")
            nc.scalar.activation(eem[:], em[:], AF.Exp)
            HB = BL // 2
            ea = [None, None]
            logc = [None, None]
            for ch in range(2):
                eac = state.tile([K, HB], f32, tag=f"ea{ch}")
                nc.vector.tensor_tensor(
                    eac[:], eem[:, 0, ch * HB:(ch + 1) * HB],
                    crf[:, 18:19].broadcast_to((K, HB)),
                    mybir.AluOpType.mult,
                )
                ea[ch] = eac
            for t_ in range(1, T):
                pea = [None, None]
                for ch in range(2):
                    pea[ch] = ps.tile([K, HB], f32, tag=f"crf{ch}", bufs=1, name=f"pea{ch}")
                    nc.tensor.matmul(pea[ch][:], crf[:, 9:9 + K], ea[ch][:],
                                     start=True, stop=True)
                for ch in range(2):
                    eaN = state.tile([K, HB], f32, tag=f"ea{ch}")
                    nc.vector.tensor_tensor(
                        eaN[:], pea[ch][:], eem[:, t_, ch * HB:(ch + 1) * HB],
                        mybir.AluOpType.mult,
                    )
                    ea[ch] = eaN
                if t_ % CRF_S == 0:
                    r = [None, None]
                    for ch in range(2):
                        r[ch] = tmp.tile([1, HB], f32, tag=f"crf_r{ch}", name=f"r{ch}")
                        nc.vector.reciprocal(r[ch][:], ea[ch][0:1, :])
                    pbc = [None, None]
                    for ch in range(2):
                        pbc[ch] = ps.tile([K, HB], f32, tag=f"crf{ch}", bufs=1, name=f"pbc{ch}")
                        nc.tensor.matmul(pbc[ch][:], crf[0:1, 23:23 + K], r[ch][:],
                                         start=True, stop=True)
                    for ch in range(2):
                        lg = tmp.tile([1, HB], f32, tag=f"crf_lg{ch}")
                        nc.scalar.activation(lg[:], ea[ch][0:1, :], AF.Ln)
                        eaN2 = state.tile([K, HB], f32, tag=f"ea{ch}")
                        nc.vector.tensor_tensor(
                            eaN2[:], ea[ch][:], pbc[ch][:], mybir.AluOpType.mult
                        )
                        logcN = state.tile([1, HB], f32, tag=f"logc{ch}")
                        if logc[ch] is None:
                            nc.vector.tensor_copy(logcN[:], lg[:])
                        else:
                            nc.vector.tensor_add(logcN[:], logc[ch][:], lg[:])
                        logc[ch] = logcN
                        ea[ch] = eaN2
            eacat = tmp.tile([K, BL], f32, tag="eacat")
            for ch in range(2):
                nc.vector.tensor_copy(eacat[:, ch * HB:(ch + 1) * HB], ea[ch][:])
            pden = ps.tile([1, BL], f32, tag="misc", bufs=1)
            nc.tensor.matmul(pden[:], crf[:, 19:20], eacat[:], start=True, stop=True)
            den = tmp.tile([1, BL], f32, tag="den")
            nc.scalar.activation(den[:], pden[:], AF.Ln)
            lcat = tmp.tile([1, BL], f32, tag="lcat")
            for ch in range(2):
                nc.vector.tensor_copy(lcat[:, ch * HB:(ch + 1) * HB], logc[ch][:])
            nc.vector.tensor_add(den[:], den[:], lcat[:])

            # ---- loss = sum_b mask_b * (den_b - num_b) ----
            diff = tmp.tile([1, BL], f32, tag="diff")
            nc.vector.tensor_sub(diff[:], den[:], num[:])
            nc.vector.tensor_mul(diff[:], diff[:], lmask[:])
            lout = tmp.tile([1, 1], f32, tag="lout")
            nc.vector.tensor_reduce(
                lout[:], diff[:], mybir.AxisListType.X, mybir.AluOpType.add
            )
            nc.sync.dma_start(loss_d[:], lout[:])

    if legalize:
        split_waits(nc)
    nc.finalize()
    return nc


def stage_inputs(inputs):
    x = np.asarray(inputs["embedding"], np.float32)
    tags = np.asarray(inputs["target_tag"]).astype(np.int64)

    def pget(name):
        return np.asarray(inputs[name], np.float32)

    def wihT(name, row_order=None):
        w = pget(name)[GATE_PERM]            # [1536, in]
        wT = w.T                             # [in, 1536]
        if row_order is not None:
            wT = wT[row_order]
        return np.ascontiguousarray(wT).reshape(-1, 128, G).astype(bf16)

    def whhT(name):
        w = pget(name)[GATE_PERM]
        return np.ascontiguousarray(w.T).reshape(NH, 128, G).astype(bf16)

    def biasv(name):
        return pget(name)[GATE_PERM].reshape(NG, 128).T

    trans, st, et = pget("trans"), pget("start_trans"), pget("end_trans")
    w_out, b_out = pget("w_out"), pget("b_out")

    in_maps = []
    for c in range(NC):
        p, par = divmod(c, 2)
        d = "f" if par == 0 else "b"
        xs = x[16 * p:16 * p + 16]
        tg = tags[16 * p:16 * p + 16]
        if par:
            xs = xs[:, ::-1]
            tg = tg[:, ::-1]
        xT_c = np.ascontiguousarray(xs.transpose(2, 1, 0)).reshape(
            NE, 128, T, BL).astype(bf16)

        w0 = wihT(f"w_ih_0{d}")
        own = np.arange(0, H) if par == 0 else np.arange(H, 2 * H)
        oth = np.arange(H, 2 * H) if par == 0 else np.arange(0, H)
        w1 = wihT(f"w_ih_1{d}", row_order=np.concatenate([own, oth]))
        whh = np.stack([whhT(f"w_hh_0{d}"), whhT(f"w_hh_1{d}")])
        bias = np.concatenate([biasv(f"b_0{d}"), biasv(f"b_1{d}")], axis=1).astype(
            np.float32)
        wh = w_out[:, 0:H] if par == 0 else w_out[:, H:2 * H]
        woutT = np.ascontiguousarray(wh.T).reshape(NH, 128, K).astype(bf16)
        bout = (b_out if par == 0 else np.zeros(K, np.float32)).reshape(K, 1)

        oh = np.zeros((K, T, BL), np.float32)
        oh[tg.T.reshape(-1), np.repeat(np.arange(T), BL), np.tile(np.arange(BL), T)] = 1.0

        tr_eff = trans if par == 0 else np.ascontiguousarray(trans.T)
        st_eff = st if par == 0 else et
        et_eff = et if par == 0 else st
        crf_c = np.zeros((K, 32), np.float32)
        crf_c[:, 0:9] = tr_eff
        crf_c[:, 9:18] = np.exp(tr_eff)
        crf_c[:, 18] = np.exp(st_eff)
        crf_c[:, 19] = np.exp(et_eff)
        crf_c[:, 20] = st_eff
        crf_c[:, 21] = et_eff
        crf_c[:, 22] = 1.0
        crf_c[0, 23:32] = 1.0
        lm = np.zeros((1, BL), np.float32)
        if par == 0:
            lm[0, 0:8] = 1.0
        else:
            lm[0, 8:16] = 1.0

        in_maps.append(
            dict(
                xT=xT_c, w0T=w0, w1T=w1, whhT=whh, bias=bias, woutT=woutT,
                bout=bout, ohT=np.ascontiguousarray(oh), crf=crf_c, lmask=lm,
                ident=np.eye(128, dtype=bf16),
            )
        )
    return in_maps


_NC_CACHE = {}


def get_nc():
    if "nc" not in _NC_CACHE:
        _NC_CACHE["nc"] = build_nc()
    return _NC_CACHE["nc"]


def kernel(**inputs):
    from concourse.bass_utils import run_bass_kernel_spmd

    nc = get_nc()
    in_maps = stage_inputs(inputs)
    res = run_bass_kernel_spmd(nc, in_maps, list(range(NC)))
    total = np.float32(0.0)
    for r in res.results:
        total += np.float32(r["loss"].reshape(-1)[0])
    return np.asarray(total, dtype=np.float32)


# revision 10
# speedup vs baseline: 1.2020x; 1.2020x over previous
"""BiLSTM(2-layer) + CRF NLL Trainium2 kernel, v2: direction-split sharding.

8 cores = 4 pairs. Pair p owns 16 sequences; core 2p runs the FORWARD
direction of both LSTM layers for those 16 sequences, core 2p+1 the BACKWARD
direction. Backward cores see time-reversed inputs, so every core runs an
identical forward-scan program; all direction asymmetry lives in host staging
(weights, reversed inputs, transposed CRF transitions, swapped start/end).

Between layers the pair exchanges hidden states with a 2-core AllGather
(bounce via DRAM, sent time-reversed so the partner receives data in its own
time order); each core reconstructs the partner's h via
(slot0 + slot1) - own, computed in fp32 so the bf16 cancellation is exact.
Emissions are per-direction partials pair-summed the same way. Each core then
runs the CRF on all 16 sequences and masks the per-sequence losses so each
sequence is counted on exactly one core.

vs v1: the serial recurrent matmul chain per core drops from 36864 LDW+MM
pairs (N=8) to 18360 (N=16), and input-projection/emission matmuls are
interleaved into the recurrence as PE filler during the per-step elementwise
tails. Gate chunks are reordered [g, i, f, o] so tanh(g)/sigmoid(i,f) start
before the step's matmuls finish and only sigmoid(o) trails them.
"""

import sys
import numpy as np
import ml_dtypes

sys.path.insert(0, "/opt/trn_rl_repo")

import concourse.bass as bass
import concourse.mybir as mybir
import concourse.tile as tile

dt = mybir.dt
AF = mybir.ActivationFunctionType
bf16 = ml_dtypes.bfloat16

# problem constants
B, T, E, H, K = 64, 256, 768, 384, 9
NC = 8
BL = 16         # sequences per core (one direction)
G = 4 * H       # 1536
NE = 6          # input contract chunks (768/128, both layers)
NH = H // 128   # 3
NG = G // 128   # 12
BLK = 32        # timesteps per xg block
NB = T // BLK   # 8
NQ = T * BL     # 4096
# CRF renorm period: measured max |ea| between renorms at S=16 is ~7e15 and
# even the worst-case growth bound stays far inside fp32 range
CRF_S = 16

# gate chunk order [i, f, g, o] = native pytorch order. The step's matmuls
# run as three PSUM-bank groups (i+f, g, o) so each activation starts as soon
# as its bank's accumulation retires, overlapping the rest of the matmul
# stream; o last so only sigmoid(o) + one mul trail the final matmul.
GATE_PERM = np.arange(4 * H)


def split_waits(nc):
    """Legalize sem waits: walrus accepts at most one sync wait per
    instruction; hoist extra waits onto same-engine NoOps."""
    import bass_rust

    n_split = 0
    for f in nc.m.functions:
        for blk in f.blocks:
            out = []
            changed = False
            for inst in blk.instructions:
                si = inst.sync_info
                if si is not None and si.on_wait and len(si.on_wait) > 1:
                    waits = list(si.on_wait)
                    for k, w in enumerate(waits[:-1]):
                        nop = mybir.InstNoOp(name=f"{inst.name}_w{k}", ins=[], outs=[])
                        nop.engine = inst.engine
                        nop.sync_info = bass_rust.SyncInfo(on_wait=[w], on_update=[])
                        out.append(nop)
                        n_split += 1
                    inst.sync_info = bass_rust.SyncInfo(
                        on_wait=[waits[-1]], on_update=list(si.on_update or [])
                    )
                    changed = True
                out.append(inst)
            if changed:
                blk.instructions = out
    return n_split


def rev_slice(a, b):
    """slice covering [a, b) traversed in reverse order."""
    return slice(b - 1, None if a == 0 else a - 1, -1)


def build_nc(legalize=True):
    nc = bass.Bass(trn_type="TRN2", num_devices=NC)
    f32 = dt.float32
    groups = [[2 * p, 2 * p + 1] for p in range(NC // 2)]

    xT_d = nc.declare_dram_parameter("xT", [NE, 128, T, BL], dt.bfloat16, False)
    w0_d = nc.declare_dram_parameter("w0T", [NE, 128, G], dt.bfloat16, False)
    w1_d = nc.declare_dram_parameter("w1T", [NE, 128, G], dt.bfloat16, False)
    whh_d = nc.declare_dram_parameter("whhT", [2, NH, 128, G], dt.bfloat16, False)
    bias_d = nc.declare_dram_parameter("bias", [128, 2 * NG], f32, False)
    wout_d = nc.declare_dram_parameter("woutT", [NH, 128, K], dt.bfloat16, False)
    bout_d = nc.declare_dram_parameter("bout", [K, 1], f32, False)
    oh_d = nc.declare_dram_parameter("ohT", [K, T, BL], f32, False)
    id_d = nc.declare_dram_parameter("ident", [128, 128], dt.bfloat16, False)
    crf_d = nc.declare_dram_parameter("crf", [K, 32], f32, False)
    lmask_d = nc.declare_dram_parameter("lmask", [1, BL], f32, False)
    loss_d = nc.declare_dram_parameter("loss", [1, 1], f32, True)

    RB = 2 * BLK  # h1 ring length (timesteps); emissions drain a block behind

    with tile.TileContext(nc) as tc:
        with (
            tc.tile_pool(name="big", bufs=1) as big,
            tc.tile_pool(name="state", bufs=2) as state,
            tc.tile_pool(name="tmp", bufs=3) as tmp,
            tc.tile_pool(name="xgp", bufs=1) as xgp,
            tc.tile_pool(name="ps", bufs=2, space="PSUM") as ps,
            tc.tile_pool(name="dram", bufs=1, space="DRAM") as dram,
        ):
            # ---- persistent loads ----
            xT = big.tile([128, NE, T, BL], dt.bfloat16, tag="xT")
            for ch in range(NE):
                nc.sync.dma_start(xT[:, ch], xT_d[ch])
            bias = big.tile([128, 2 * NG], f32, tag="bias")
            nc.sync.dma_start(bias[:], bias_d[:])
            wout = big.tile([128, NH, K], dt.bfloat16, tag="wout")
            for ch in range(NH):
                nc.sync.dma_start(wout[:, ch], wout_d[ch])
            bout = big.tile([K, 1], f32, tag="bout")
            nc.sync.dma_start(bout[:], bout_d[:])
            ident = big.tile([128, 128], dt.bfloat16, tag="ident")
            nc.sync.dma_start(ident[:], id_d[:])
            # stage via DVE copies (single-DMA-queue-consumer rule)
            ohT_raw = big.tile([K, T, BL], f32, tag="em")  # slot later: u, em
            nc.sync.dma_start(ohT_raw[:], oh_d[:])
            ohT = big.tile([K, T, BL], f32, tag="ohT")
            nc.vector.tensor_copy(ohT[:], ohT_raw[:])
            crf_raw = big.tile([K, 32], f32, tag="crf_raw")
            nc.sync.dma_start(crf_raw[:], crf_d[:])
            crf = big.tile([K, 32], f32, tag="crf")
            nc.vector.tensor_copy(crf[:], crf_raw[:])
            lmask_raw = big.tile([1, BL], f32, tag="lmask_raw")
            nc.sync.dma_start(lmask_raw[:], lmask_d[:])
            lmask = big.tile([1, BL], f32, tag="lmask")
            nc.vector.tensor_copy(lmask[:], lmask_raw[:])

            h0 = big.tile([128, NH, T, BL], dt.bfloat16, tag="h0")
            h1r = big.tile([128, NH, RB, BL], dt.bfloat16, tag="h1r")

            # DRAM bounce buffers for the pairwise exchanges. h0 is exchanged
            # in NS pipelined segments, each fired as soon as its blocks are
            # computed; only the last segment's latency is exposed at the
            # layer boundary.
            NS = 4
            SEG = T // NS
            b0_in = [
                dram.tile([128, NH, SEG, BL], dt.bfloat16, tag=f"b0_in{s}",
                          name=f"b0_in{s}")
                for s in range(NS)
            ]
            b0_out = [
                nc.dram_tensor(f"b0_out{s}", [2, 128, NH, SEG, BL], dt.bfloat16)
                for s in range(NS)
            ]
            b1_in = dram.tile([K, T, BL], f32, tag="b1_in")
            b1_out = nc.dram_tensor("b1_out", [2, K, T, BL], f32)

            em = None  # allocated after the h0 exchange (shares slot with u)

            # numerator transition-pairs scratch: filled one tile per layer-0
            # block (depends only on ohT/crf), reduced at layer-0 end
            oh_flat = ohT[:].rearrange("k t b -> k (t b)")
            NTC = 512
            NQm = NQ - BL
            scr2 = big.tile([K, BL, T], f32, tag="XC")
            scr2_tb = scr2[:].rearrange("k b t -> k t b")  # [K, T, BL]
            tr_t = tmp.tile([K, BL], f32, tag="trt")

            def pairs_task(nt):
                n0 = nt * NTC
                n1 = min(n0 + NTC, NQm)
                pa = ps.tile([K, NTC], f32, tag="misc", bufs=1, name=f"pa{nt}")
                nc.tensor.matmul(pa[:, 0:n1 - n0], crf[:, 0:K], oh_flat[:, n0:n1],
                                 start=True, stop=True)
                nc.vector.tensor_tensor(
                    scr2_tb[:, n0 // BL:n1 // BL, :],
                    pa[:, 0:n1 - n0], oh_flat[:, n0 + BL:n1 + BL],
                    mybir.AluOpType.mult,
                )
                if nt == NB - 1:
                    nc.vector.tensor_reduce(
                        tr_t[:], scr2[:, :, 0:T - 1], mybir.AxisListType.X,
                        mybir.AluOpType.add,
                    )

            class XgEmitter:
                """Incrementally emits the input-projection matmuls for one
                32-step block (12 gate chunks x 6 contract chunks) so they can
                be interleaved into the recurrence as PE filler."""

                def __init__(self, layer, blk, xg_tile, wih):
                    self.layer, self.blk, self.xg, self.wih = layer, blk, xg_tile, wih
                    self.j, self.kc, self.p = 0, 0, None

                def rhs(self, kc):
                    sl = slice(self.blk * BLK, (self.blk + 1) * BLK)
                    if self.layer == 0:
                        return xT[:, kc, sl, :]
                    if kc < NH:
                        return h0[:, kc, sl, :]
                    return xT[:, kc - NH, sl, :]  # partner h0 lives in xT[:, 0:3]

                def step(self):
                    if self.j >= NG:
                        return False
                    if self.kc == 0:
                        self.p = ps.tile([128, BLK * BL], dt.float32, tag="pxg", bufs=2)
                    j = self.j
                    nc.tensor.matmul(
                        self.p[:],
                        self.wih[:, self.kc, j * 128:(j + 1) * 128],
                        self.rhs(self.kc),
                        start=(self.kc == 0),
                        stop=(self.kc == NE - 1),
                    )
                    self.kc += 1
                    if self.kc == NE:
                        bcol = self.layer * NG + j
                        nc.scalar.add(self.xg[:, j], self.p[:], bias[:, bcol:bcol + 1])
                        self.kc = 0
                        self.j += 1
                    return True

                def drain(self):
                    while self.step():
                        pass

            # ---- two LSTM layers (one direction each; SPMD over cores) ----
            for layer in range(2):
                wih = big.tile([128, NE, G], dt.bfloat16, tag="wih")
                w_src = w0_d if layer == 0 else w1_d
                for ch in range(NE):
                    nc.sync.dma_start(wih[:, ch], w_src[ch])
                whh = big.tile([128, NH, G], dt.bfloat16, tag="whh")
                for kc in range(NH):
                    nc.sync.dma_start(whh[:, kc], whh_d[layer, kc])

                if layer == 1:
                    em = big.tile([K, T, BL], f32, tag="em")

                def h_chunk(t, kc):
                    if layer == 0:
                        return h0[:, kc, t, :]
                    return h1r[:, kc, t % RB, :]

                def h_full(t):
                    if layer == 0:
                        return h0[:, :, t, :]
                    return h1r[:, :, t % RB, :]

                xg_cur = xgp.tile([128, NG, BLK * BL], dt.bfloat16, tag="xg", bufs=2)
                em0 = XgEmitter(layer, 0, xg_cur, wih)
                em0.drain()

                c_st = None
                for blk in range(NB):
                    if blk + 1 < NB:
                        xg_nxt = xgp.tile(
                            [128, NG, BLK * BL], dt.bfloat16, tag="xg", bufs=2
                        )
                        nxt = XgEmitter(layer, blk + 1, xg_nxt, wih)
                    else:
                        xg_nxt, nxt = None, None

                    for tl in range(BLK):
                        t = blk * BLK + tl
                        first = t == 0
                        u0 = tl * BL

                        # Gate pre-activations land in three separate PSUM
                        # banks (i+f, g, o). Each bank's group: recurrent
                        # whh matmuls plus one identity-matmul per gate chunk
                        # that injects xg (incl. bias) straight into PSUM —
                        # no DVE pre-adds, and each activation reads its bank
                        # as soon as that group retires while the PE streams
                        # the next group.
                        gp_if = ps.tile([128, 2 * NH, BL], f32, tag="gp_if", bufs=1)
                        gp_g = ps.tile([128, NH, BL], f32, tag="gp_g", bufs=1)
                        gp_o = ps.tile([128, NH, BL], f32, tag="gp_o", bufs=1)

                        def emit_group(tile_, j0, nj):
                            # ONE identity matmul FIRST (start=True): it has
                            # no dependency on h(t-1), so it issues during the
                            # previous step's elementwise tail and injects xg
                            # for the whole group; the nj*NH recurrent matmuls
                            # then accumulate on top. kc-outer order: the
                            # first matmuls need only h chunk 0, which the
                            # chunk-split hmul below writes first.
                            total = (0 if first else nj * NH) + 1
                            nc.tensor.matmul(
                                tile_[:],
                                ident[:],
                                xg_cur[:, j0:j0 + nj, u0:u0 + BL],
                                start=True,
                                stop=(total == 1),
                            )
                            n = 1
                            if not first:
                                for jj in range(nj):
                                    j = j0 + jj
                                    for kc in range(NH):
                                        nc.tensor.matmul(
                                            tile_[:, jj],
                                            whh[:, kc, j * 128:(j + 1) * 128],
                                            h_chunk(t - 1, kc),
                                            start=False,
                                            stop=(n == total - 1),
                                        )
                                        n += 1

                        emit_group(gp_if, 0, 2 * NH)
                        emit_group(gp_g, 2 * NH, NH)
                        emit_group(gp_o, 3 * NH, NH)

                        sif = tmp.tile([128, 2 * NH, BL], f32, tag="sif")
                        nc.scalar.activation(sif[:], gp_if[:], AF.Sigmoid)
                        tg = tmp.tile([128, NH, BL], f32, tag="tg")
                        nc.scalar.activation(tg[:], gp_g[:], AF.Tanh)
                        so = tmp.tile([128, NH, BL], f32, tag="so")
                        nc.scalar.activation(so[:], gp_o[:], AF.Sigmoid)

                        cN = state.tile([128, NH, BL], f32, tag="c")
                        if first:
                            nc.vector.tensor_mul(cN[:], sif[:, 0:NH], tg[:])
                        else:
                            t2 = tmp.tile([128, NH, BL], f32, tag="t2")
                            nc.vector.tensor_mul(t2[:], sif[:, NH:2 * NH], c_st[:])
                            t1 = tmp.tile([128, NH, BL], f32, tag="t1")
                            nc.vector.tensor_mul(t1[:], sif[:, 0:NH], tg[:])
                            nc.vector.tensor_add(cN[:], t1[:], t2[:])
                        c_st = cN

                        tc_t = tmp.tile([128, NH, BL], f32, tag="tc")
                        nc.scalar.activation(tc_t[:], cN[:], AF.Tanh)
                        nc.vector.tensor_mul(h_full(t), so[:], tc_t[:])

                        # PE filler: next block's input projections, spread
                        # evenly so every step's elementwise tail is covered
                        # (72 matmuls over 32 steps = 2-3 per step)
                        if nxt is not None:
                            nmm = NG * NE
                            quota = ((tl + 1) * nmm) // BLK - (tl * nmm) // BLK
                            for _ in range(quota):
                                nxt.step()

                    if nxt is not None:
                        nxt.drain()
                        xg_cur = xg_nxt

                    if layer == 0:
                        # send this h0 block time-reversed into its segment
                        # bounce (per chunk: DMA APs are limited to 3 dims)
                        seg = blk // (BLK_PER_SEG := NB // NS)
                        t1r = SEG * (seg + 1) - blk * BLK
                        rsl = rev_slice(t1r - BLK, t1r)
                        for c in range(NH):
                            nc.sync.dma_start(
                                b0_in[seg][:, c, rsl, :],
                                h0[:, c, blk * BLK:(blk + 1) * BLK, :],
                            )
                        pairs_task(blk)
                        if blk % BLK_PER_SEG == BLK_PER_SEG - 1 and seg < NS - 1:
                            # segment collective fires as soon as its blocks
                            # are sent; transfer hides under remaining compute
                            nc.gpsimd.collective_compute(
                                "AllGather",
                                mybir.AluOpType.bypass,
                                replica_groups=groups,
                                ins=[b0_in[seg][:].opt()],
                                outs=[b0_out[seg][:].opt()],
                            )
                    else:
                        # emissions for the ring block just completed
                        r0 = (blk % 2) * BLK
                        pem = ps.tile([K, BLK, BL], f32, tag="misc", bufs=1)
                        for kc in range(NH):
                            nc.tensor.matmul(
                                pem[:],
                                wout[:, kc],
                                h1r[:, kc, r0:r0 + BLK, :],
                                start=(kc == 0),
                                stop=(kc == NH - 1),
                            )
                        nc.scalar.add(
                            em[:, blk * BLK:(blk + 1) * BLK, :], pem[:], bout[:, 0:1]
                        )

                if layer == 0:
                    # ---- last h0 segment exchange (only this one's latency
                    # is exposed) ----
                    nc.gpsimd.collective_compute(
                        "AllGather",
                        mybir.AluOpType.bypass,
                        replica_groups=groups,
                        ins=[b0_in[NS - 1][:].opt()],
                        outs=[b0_out[NS - 1][:].opt()],
                    )
                    # partner h0 = (slot0 + slot1) - own(reversed); fp32 sum
                    # makes the bf16 cancellation exact. Descending segments:
                    # the last segment holds partner t 0.., consumed first.
                    for s in range(NS - 1, -1, -1):
                        lo = T - SEG * (s + 1)
                        for c in range(NH):
                            s0c = big.tile([128, SEG, BL], dt.bfloat16, tag="XC")
                            nc.sync.dma_start(s0c[:], b0_out[s][0, :, c])
                            s1c = big.tile([128, SEG, BL], dt.bfloat16, tag="XD")
                            nc.sync.dma_start(s1c[:], b0_out[s][1, :, c])
                            u = big.tile([128, SEG, BL], f32, tag="em")
                            nc.vector.tensor_add(u[:], s0c[:], s1c[:])
                            nc.vector.tensor_sub(
                                xT[:, c, lo:lo + SEG, :], u[:],
                                h0[:, c, rev_slice(T - lo - SEG, T - lo), :],
                            )

            # ---- emissions exchange: em_full = own partial + partner partial ----
            nc.sync.dma_start(b1_in[:, ::-1, :], em[:])
            nc.gpsimd.collective_compute(
                "AllGather",
                mybir.AluOpType.bypass,
                replica_groups=groups,
                ins=[b1_in[:].opt()],
                outs=[b1_out[:].opt()],
            )
            s0e = big.tile([K, T, BL], f32, tag="wih")
            nc.sync.dma_start(s0e[:], b1_out[0])
            s1e = big.tile([K, T, BL], f32, tag="XC")
            nc.sync.dma_start(s1e[:], b1_out[1])
            nc.vector.tensor_add(s0e[:], s0e[:], s1e[:])
            # em_full = own + partner. The slot sum minus our (reversed) send
            # gives the PARTNER partial in our time order; our own partial
            # must be added back. (Written to a fresh tile: a reversed
            # self-read in one op would race an in-place write.)
            em2 = big.tile([K, T, BL], f32, tag="whh")
            nc.vector.tensor_sub(em2[:], s0e[:], em[:, ::-1, :])
            nc.vector.tensor_add(em2[:], em2[:], em[:])
            em = em2

            # ---- gold path score (numerator) ----
            scr = big.tile([K, BL, T], f32, tag="wih")
            nkb = tmp.tile([K, BL], f32, tag="nkb")
            nc.vector.tensor_tensor(
                scr[:].rearrange("k b t -> k t b"),
                em[:], ohT[:], mybir.AluOpType.mult,
            )
            nc.vector.tensor_reduce(
                nkb[:], scr[:], mybir.AxisListType.X, mybir.AluOpType.add
            )
            nc.vector.tensor_add(nkb[:], nkb[:], tr_t[:])
            pnum = ps.tile([1, BL], f32, tag="misc", bufs=1)
            nc.tensor.matmul(pnum[:], crf[:, 22:23], nkb[:], start=True, stop=False)
            nc.tensor.matmul(pnum[:], crf[:, 20:21], ohT[:, 0, :], start=False, stop=False)
            nc.tensor.matmul(pnum[:], crf[:, 21:22], ohT[:, T - 1, :], start=False, stop=True)
            num = tmp.tile([1, BL], f32, tag="num")
            nc.vector.tensor_copy(num[:], pnum[:])

            # ---- CRF forward algorithm (denominator), linear space ----
            # two independent 8-sequence chains interleaved so each chain's
            # PE->DVE latency hides under the other's ops
            eem = big.tile([K, T, BL], f32, tag="wih")
            nc.scalar.activation(eem[:], em[:], AF.Exp)
            HB = BL // 2
            ea = [None, None]
            logc = [None, None]
            for ch in range(2):
                eac = state.tile([K, HB], f32, tag=f"ea{ch}")
                nc.vector.tensor_tensor(
                    eac[:], eem[:, 0, ch * HB:(ch + 1) * HB],
                    crf[:, 18:19].broadcast_to((K, HB)),
                    mybir.AluOpType.mult,
                )
                ea[ch] = eac
            for t_ in range(1, T):
                pea = [None, None]
                for ch in range(2):
                    pea[ch] = ps.tile([K, HB], f32, tag=f"crf{ch}", bufs=1, name=f"pea{ch}")
                    nc.tensor.matmul(pea[ch][:], crf[:, 9:9 + K], ea[ch][:],
                                     start=True, stop=True)
                for ch in range(2):
                    eaN = state.tile([K, HB], f32, tag=f"ea{ch}")
                    nc.vector.tensor_tensor(
                        eaN[:], pea[ch][:], eem[:, t_, ch * HB:(ch + 1) * HB],
                        mybir.AluOpType.mult,
                    )
                    ea[ch] = eaN
                if t_ % CRF_S == 0:
                    r = [None, None]
                    for ch in range(2):
                        r[ch] = tmp.tile([1, HB], f32, tag=f"crf_r{ch}", name=f"r{ch}")
                        nc.vector.reciprocal(r[ch][:], ea[ch][0:1, :])
                    pbc = [None, None]
                    for ch in range(2):
                        pbc[ch] = ps.tile([K, HB], f32, tag=f"crf{ch}", bufs=1, name=f"pbc{ch}")
                        nc.tensor.matmul(pbc[ch][:], crf[0:1, 23:23 + K], r[ch][:],
                                         start=True, stop=True)
                    for ch in range(2):
                        lg = tmp.tile([1, HB], f32, tag=f"crf_lg{ch}")
                        nc.scalar.activation(lg[:], ea[ch][0:1, :], AF.Ln)
                        eaN2 = state.tile([K, HB], f32, tag=f"ea{ch}")
                        nc.vector.tensor_tensor(
                            eaN2[:], ea[ch][:], pbc[ch][:], mybir.AluOpType.mult
                        )
                        logcN = state.tile([1, HB], f32, tag=f"logc{ch}")
                        if logc[ch] is None:
                            nc.vector.tensor_copy(logcN[:], lg[:])
                        else:
                            nc.vector.tensor_add(logcN[:], logc[ch][:], lg[:])
                        logc[ch] = logcN
                        ea[ch] = eaN2
            eacat = tmp.tile([K, BL], f32, tag="eacat")
            for ch in range(2):
                nc.vector.tensor_copy(eacat[:, ch * HB:(ch + 1) * HB], ea[ch][:])
            pden = ps.tile([1, BL], f32, tag="misc", bufs=1)
            nc.tensor.matmul(pden[:], crf[:, 19:20], eacat[:], start=True, stop=True)
            den = tmp.tile([1, BL], f32, tag="den")
            nc.scalar.activation(den[:], pden[:], AF.Ln)
            lcat = tmp.tile([1, BL], f32, tag="lcat")
            for ch in range(2):
                nc.vector.tensor_copy(lcat[:, ch * HB:(ch + 1) * HB], logc[ch][:])
            nc.vector.tensor_add(den[:], den[:], lcat[:])

            # ---- loss = sum_b mask_b * (den_b - num_b) ----
            diff = tmp.tile([1, BL], f32, tag="diff")
            nc.vector.tensor_sub(diff[:], den[:], num[:])
            nc.vector.tensor_mul(diff[:], diff[:], lmask[:])
            lout = tmp.tile([1, 1], f32, tag="lout")
            nc.vector.tensor_reduce(
                lout[:], diff[:], mybir.AxisListType.X, mybir.AluOpType.add
            )
            nc.sync.dma_start(loss_d[:], lout[:])

    if legalize:
        split_waits(nc)
    nc.finalize()
    return nc


def stage_inputs(inputs):
    x = np.asarray(inputs["embedding"], np.float32)
    tags = np.asarray(inputs["target_tag"]).astype(np.int64)

    def pget(name):
        return np.asarray(inputs[name], np.float32)

    def wihT(name, row_order=None):
        w = pget(name)[GATE_PERM]            # [1536, in]
        wT = w.T                             # [in, 1536]
        if row_order is not None:
            wT = wT[row_order]
        return np.ascontiguousarray(wT).reshape(-1, 128, G).astype(bf16)

    def whhT(name):
        w = pget(name)[GATE_PERM]
        return np.ascontiguousarray(w.T).reshape(NH, 128, G).astype(bf16)

    def biasv(name):
        return pget(name)[GATE_PERM].reshape(NG, 128).T

    trans, st, et = pget("trans"), pget("start_trans"), pget("end_trans")
    w_out, b_out = pget("w_out"), pget("b_out")

    in_maps = []
    for c in range(NC):
        p, par = divmod(c, 2)
        d = "f" if par == 0 else "b"
        xs = x[16 * p:16 * p + 16]
        tg = tags[16 * p:16 * p + 16]
        if par:
            xs = xs[:, ::-1]
            tg = tg[:, ::-1]
        xT_c = np.ascontiguousarray(xs.transpose(2, 1, 0)).reshape(
            NE, 128, T, BL).astype(bf16)

        w0 = wihT(f"w_ih_0{d}")
        own = np.arange(0, H) if par == 0 else np.arange(H, 2 * H)
        oth = np.arange(H, 2 * H) if par == 0 else np.arange(0, H)
        w1 = wihT(f"w_ih_1{d}", row_order=np.concatenate([own, oth]))
        whh = np.stack([whhT(f"w_hh_0{d}"), whhT(f"w_hh_1{d}")])
        bias = np.concatenate([biasv(f"b_0{d}"), biasv(f"b_1{d}")], axis=1).astype(
            np.float32)
        wh = w_out[:, 0:H] if par == 0 else w_out[:, H:2 * H]
        woutT = np.ascontiguousarray(wh.T).reshape(NH, 128, K).astype(bf16)
        bout = (b_out if par == 0 else np.zeros(K, np.float32)).reshape(K, 1)

        oh = np.zeros((K, T, BL), np.float32)
        oh[tg.T.reshape(-1), np.repeat(np.arange(T), BL), np.tile(np.arange(BL), T)] = 1.0

        tr_eff = trans if par == 0 else np.ascontiguousarray(trans.T)
        st_eff = st if par == 0 else et
        et_eff = et if par == 0 else st
        crf_c = np.zeros((K, 32), np.float32)
        crf_c[:, 0:9] = tr_eff
        crf_c[:, 9:18] = np.exp(tr_eff)
        crf_c[:, 18] = np.exp(st_eff)
        crf_c[:, 19] = np.exp(et_eff)
        crf_c[:, 20] = st_eff
        crf_c[:, 21] = et_eff
        crf_c[:, 22] = 1.0
        crf_c[0, 23:32] = 1.0
        lm = np.zeros((1, BL), np.float32)
        if par == 0:
            lm[0, 0:8] = 1.0
        else:
            lm[0, 8:16] = 1.0

        in_maps.append(
            dict(
                xT=xT_c, w0T=w0, w1T=w1, whhT=whh, bias=bias, woutT=woutT,
                bout=bout, ohT=np.ascontiguousarray(oh), crf=crf_c, lmask=lm,
                ident=np.eye(128, dtype=bf16),
            )
        )
    return in_maps


_NC_CACHE = {}


def get_nc():
    if "nc" not in _NC_CACHE:
        _NC_CACHE["nc"] = build_nc()
    return _NC_CACHE["nc"]


def kernel(**inputs):
    from concourse.bass_utils import run_bass_kernel_spmd

    nc = get_nc()
    in_maps = stage_inputs(inputs)
    res = run_bass_kernel_spmd(nc, in_maps, list(range(NC)))
    total = np.float32(0.0)
    for r in res.results:
        total += np.float32(r["loss"].reshape(-1)[0])
    return np.asarray(total, dtype=np.float32)
